# revision 25
# baseline (speedup 1.0000x reference)
"""CompressKV Trainium2 kernel.

Reference computation (see problem):
  kv [32768, 2, 8, 128] fp32, 4 sequences of 8192 tokens.
  Each sequence is cut into 511 chunks of 32 tokens, stride 16.
  compress_k[l,h,o] = sum_{t,i} k[chunk l, t, h, i] * k_conv_w[o,i,t] + k_conv_b[o]
  (same for v), returns (compress_k, compress_v, cu_comp).

Strategy:
  - Head-sharded across the 8 NeuronCores (weights are shared across heads,
    each core handles one head for both K and V over all 4 sequences).
  - Chunks overlap by half (kernel 32, stride 16) -> block decomposition:
    block b = tokens [16b, 16b+16); out[l] = Gf[l] + Gs[l+1] where
    Gf[b] = W[:, :, 0:16] . block_b and Gs[b] = W[:, :, 16:32] . block_b.
    This way each token is streamed from HBM exactly once.
  - Per (kv, seq): 32 accumulating matmuls of [K=128 (=head_dim), M=128 (=out)]
    x [K=128, N=512 blocks] into two PSUM banks (Gf, Gs). Host pre-transposes
    the data so every DMA and every matmul access is contiguous.
  - float32r matmul dtype: full-rate (1 cycle/row) fp32 streaming mode.
    Internally the PE rounds both operands to ~11 mantissa bits (tf32-like);
    measured end-to-end L2 relative error ~4e-4 vs the fp32 reference.
  - Shift-add (Gf[l] + Gs[l+1]) + bias done on ACT + DVE, exact fp32.
"""
import sys

sys.path.insert(0, "/opt/trn_rl_repo")

import numpy as np

from concourse import bass, bacc, tile, bass_utils, mybir

B = 4          # sequences
SEQ = 8192     # tokens per sequence
H = 8          # heads (== number of cores)
D = 128        # head dim
KS = 32        # chunk size
STRIDE = 16    # chunk stride
NB = SEQ // STRIDE          # 512 blocks per sequence
NCH = (SEQ - KS) // STRIDE + 1  # 511 chunks per sequence
NT = STRIDE    # 16 K-tiles per conv half
L = B * NCH    # 2044 chunks total

F32 = mybir.dt.float32
F32R = mybir.dt.float32r
F16 = mybir.dt.float16

_compiled = {}

# results of the last hardware run (for test harness introspection)
last_results = None


TQ = 8           # conv taps per x DMA unit (1 MB per transfer)
NQ = NT // TQ    # 4 quarters per (kv, seq)


def _build(x_bufs=13):
    nc = bacc.Bacc("TRN2", target_bir_lowering=False, debug=False, num_devices=H)

    # per-core inputs (core h owns head h)
    # x: [kv, seq, i, t', block]: for any t'-range, each partition row (fixed i)
    #    is contiguous in DRAM -> DMA descriptors are multi-KB runs
    x_dram = nc.dram_tensor("x", [2, B, D, NT, NB], F16, kind="ExternalInput")
    # w: chunk (kv, tq) holds lhsT tiles for taps [tq*TQ, tq*TQ+TQ) of the
    #    first conv half (Gf) followed by the same taps of the second (Gs)
    w_dram = nc.dram_tensor("w", [2, NQ, D, 2 * TQ * D], F16, kind="ExternalInput")
    bias_dram = nc.dram_tensor("bias", [D, 2], F32, kind="ExternalInput")
    # out: [kv, seq, o, chunk]
    out_dram = nc.dram_tensor("out", [2, B, D, NCH], F32, kind="ExternalOutput")

    # DMA ring discipline: nc.sync ring carries the x stream and the weight
    # chunks in exact consumption order (HWDGE completes FIFO per ring, so
    # the next-needed slab always lands first); bias/stores on nc.scalar.
    with tile.TileContext(nc) as tc:
        with (
            tc.tile_pool(name="sbw", bufs=1) as sbw,
            tc.tile_pool(name="sbx", bufs=x_bufs) as sbx,
            tc.tile_pool(name="sbo", bufs=6) as sbo,
            tc.tile_pool(name="ps", bufs=3, space=bass.MemorySpace.PSUM) as ps,
            tc.tile_pool(name="dram", bufs=1, space="DRAM") as dram,
        ):
            bias_sb = sbw.tile([D, 2], F32)
            nc.scalar.dma_start(bias_sb[:], bias_dram[:])

            # PE warm-up: a dozen matmuls on memset tiles run during the DMA
            # ramp (no data dependency), so the HAM clock-gate reaches 2.4 GHz
            # before the first real matmul. Result is sunk to a DRAM scratch
            # tile so nothing can dead-code it.
            warm_w = sbw.tile([D, D], F16)
            warm_x = sbw.tile([D, NB], F16)
            nc.gpsimd.memset(warm_w[:], 0.0)
            nc.gpsimd.memset(warm_x[:], 0.0)
            warm_ps = ps.tile([D, NB], F32, tag="warm", bufs=1)
            N_WARM = 12
            for i in range(N_WARM):
                nc.tensor.matmul(warm_ps[:], warm_w[:], warm_x[:],
                                 start=(i == 0), stop=(i == N_WARM - 1))
            warm_out = sbo.tile([D, NB], F32, tag="warmout", bufs=1)
            nc.vector.tensor_copy(warm_out[:], warm_ps[:])
            warm_sink = dram.tile([D, NB], F32)
            nc.scalar.dma_start(warm_sink[:], warm_out[:])

            # weight chunks ride the sync ring just-in-time, in consumption
            # order; the first two taps of chunk (0,0) go first as tiny
            # transfers so the PE unblocks early (Tile tracks subtile deps)
            w_sb = {}

            def ensure_w(c, tqc):
                if (c, tqc) in w_sb:
                    return w_sb[(c, tqc)]
                wt = sbw.tile([D, 2 * TQ * D], F16, name=f"w_sb_{c}_{tqc}")
                w_sb[(c, tqc)] = wt
                if c == 0 and tqc == 0:
                    nc.sync.dma_start(wt[:, 0:2 * D], w_dram[c, tqc, :, 0:2 * D])
                    nc.sync.dma_start(
                        wt[:, TQ * D:(TQ + 2) * D],
                        w_dram[c, tqc, :, TQ * D:(TQ + 2) * D],
                    )
                else:
                    nc.sync.dma_start(wt[:], w_dram[c, tqc])
                return wt

            def finish_head_w(wt):
                # remainder of chunk (0,0), issued after the first x sub-unit
                nc.sync.dma_start(wt[:, 2 * D:TQ * D],
                                  w_dram[0, 0, :, 2 * D:TQ * D])
                nc.sync.dma_start(wt[:, (TQ + 2) * D:],
                                  w_dram[0, 0, :, (TQ + 2) * D:])

            for c in range(2):
                for s in range(B):
                    gf = ps.tile([D, NB], F32, tag="gf", name=f"gf_{c}_{s}")
                    gs = ps.tile([D, NB], F32, tag="gs", name=f"gs_{c}_{s}")
                    # ramp the very first unit so the PE starts early
                    if c == 0 and s == 0:
                        ranges = [(0, 2), (2, 4), (4, 8), (8, 16)]
                    else:
                        ranges = [(0, TQ), (TQ, 2 * TQ)][:NT // TQ]
                    for (ta, tb) in ranges:
                        tqc = ta // TQ
                        wid = tb - ta
                        wt = ensure_w(c, tqc)
                        xt = sbx.tile([D, wid, NB], F16, tag=f"x{wid}",
                                      bufs=(None if wid == TQ else 2),
                                      name=f"x_{c}_{s}_{ta}")
                        nc.sync.dma_start(xt[:], x_dram[c, s, :, ta:tb, :])
                        if c == 0 and s == 0 and ta == 0:
                            finish_head_w(wt)
                        for lt in range(wid):
                            t = ta + lt
                            wo = t - tqc * TQ
                            nc.tensor.matmul(
                                gf[:], wt[:, wo * D:(wo + 1) * D], xt[:, lt, :],
                                start=(t == 0), stop=(t == NT - 1),
                            )
                            nc.tensor.matmul(
                                gs[:], wt[:, (TQ + wo) * D:(TQ + wo + 1) * D],
                                xt[:, lt, :],
                                start=(t == 0), stop=(t == NT - 1),
                            )
                    # out[:, l] = Gf[:, l] + Gs[:, l+1] + bias, l in [0, 511)
                    t1 = sbo.tile([D, NCH], F32, tag="t1", name=f"t1_{c}_{s}")
                    nc.scalar.activation(
                        t1[:], gf[:, 0:NCH],
                        mybir.ActivationFunctionType.Identity,
                        bias=bias_sb[:, c:c + 1],
                    )
                    ot = sbo.tile([D, NCH], F32, tag="ot", name=f"ot_{c}_{s}")
                    nc.vector.tensor_add(ot[:], t1[:], gs[:, 1:NB])
                    nc.scalar.dma_start(out_dram[c, s], ot[:])

    nc.compile()
    return nc


def _get_nc():
    if "nc" not in _compiled:
        _compiled["nc"] = _build()
    return _compiled["nc"]


def kernel(kv, cu_seqlens, k_conv_w, k_conv_b, v_conv_w, v_conv_b):
    global last_results
    kv = np.ascontiguousarray(np.asarray(kv, dtype=np.float32))
    k_conv_w = np.asarray(k_conv_w, dtype=np.float32)
    v_conv_w = np.asarray(v_conv_w, dtype=np.float32)
    k_conv_b = np.asarray(k_conv_b, dtype=np.float32)
    v_conv_b = np.asarray(v_conv_b, dtype=np.float32)

    nc = _get_nc()

    # x view: [s, b, t, c, h, i]
    a = kv.reshape(B, NB, NT, 2, H, D)
    # weights: [o, i, t] -> [c, i, t, o] -> per-quarter chunks
    # chunk (c, tq) = taps [tq*TQ, (tq+1)*TQ) of half 0 ++ same taps of half 1
    wf = np.stack([k_conv_w, v_conv_w]).transpose(0, 2, 3, 1)  # [2, i, t, o]
    w_all = np.ascontiguousarray(
        np.stack(
            [
                np.concatenate(
                    [
                        wf[:, :, tq * TQ:(tq + 1) * TQ, :],
                        wf[:, :, NT + tq * TQ:NT + (tq + 1) * TQ, :],
                    ],
                    axis=2,
                )
                for tq in range(NQ)
            ],
            axis=1,
        )
    ).reshape(2, NQ, D, 2 * TQ * D).astype(np.float16)  # [2, tq, i, (tap, o)]
    bias_all = np.ascontiguousarray(
        np.stack([k_conv_b, v_conv_b], axis=1)
    )  # [o, 2]

    in_maps = []
    for h in range(H):
        xh = np.ascontiguousarray(
            a[:, :, :, :, h, :].transpose(3, 0, 4, 2, 1).astype(np.float16)
        )  # [c, s, i, t, b]
        in_maps.append({"x": xh, "w": w_all, "bias": bias_all})

    res = bass_utils.run_bass_kernel_spmd(
        nc, in_maps, core_ids=list(range(H)),
    )
    last_results = res

    compress_k = np.empty((L, H, D), dtype=np.float32)
    compress_v = np.empty((L, H, D), dtype=np.float32)
    for h in range(H):
        out = res.results[h]["out"]  # [2, B, D, NCH]
        compress_k[:, h, :] = out[0].transpose(0, 2, 1).reshape(L, D)
        compress_v[:, h, :] = out[1].transpose(0, 2, 1).reshape(L, D)
    cu_comp = np.arange(B + 1, dtype=np.int32) * NCH
    return compress_k, compress_v, cu_comp


# revision 28
# speedup vs baseline: 1.2608x; 1.2608x over previous
"""CompressKV Trainium2 kernel.

Reference computation (see problem):
  kv [32768, 2, 8, 128] fp32, 4 sequences of 8192 tokens.
  Each sequence is cut into 511 chunks of 32 tokens, stride 16.
  compress_k[l,h,o] = sum_{t,i} k[chunk l, t, h, i] * k_conv_w[o,i,t] + k_conv_b[o]
  (same for v), returns (compress_k, compress_v, cu_comp).

Strategy:
  - Head-sharded across the 8 NeuronCores (weights are shared across heads,
    each core handles one head for both K and V over all 4 sequences).
  - Chunks overlap by half (kernel 32, stride 16) -> block decomposition:
    block b = tokens [16b, 16b+16); out[l] = Gf[l] + Gs[l+1] where
    Gf[b] = W[:, :, 0:16] . block_b and Gs[b] = W[:, :, 16:32] . block_b.
    This way each token is streamed from HBM exactly once.
  - Per (kv, seq): 32 accumulating matmuls of [K=128 (=head_dim), M=128 (=out)]
    x [K=128, N=512 blocks] into two PSUM banks (Gf, Gs). Host pre-transposes
    the data so every DMA and every matmul access is contiguous.
  - float32r matmul dtype: full-rate (1 cycle/row) fp32 streaming mode.
    Internally the PE rounds both operands to ~11 mantissa bits (tf32-like);
    measured end-to-end L2 relative error ~4e-4 vs the fp32 reference.
  - Shift-add (Gf[l] + Gs[l+1]) + bias done on ACT + DVE, exact fp32.
"""
import sys

sys.path.insert(0, "/opt/trn_rl_repo")

import numpy as np

from concourse import bass, bacc, tile, bass_utils, mybir

B = 4          # sequences
SEQ = 8192     # tokens per sequence
H = 8          # heads (== number of cores)
D = 128        # head dim
KS = 32        # chunk size
STRIDE = 16    # chunk stride
NB = SEQ // STRIDE          # 512 blocks per sequence
NCH = (SEQ - KS) // STRIDE + 1  # 511 chunks per sequence
NT = STRIDE    # 16 K-tiles per conv half
L = B * NCH    # 2044 chunks total

F32 = mybir.dt.float32
F32R = mybir.dt.float32r
F16 = mybir.dt.float16

_compiled = {}

# results of the last hardware run (for test harness introspection)
last_results = None


TQ = 8           # conv taps per x DMA unit (1 MB per transfer)
NQ = NT // TQ    # 4 quarters per (kv, seq)


def _build(x_bufs=13):
    nc = bacc.Bacc("TRN2", target_bir_lowering=False, debug=False, num_devices=H)

    # per-core inputs (core h owns head h)
    # x: [kv, seq, i, t', block]: for any t'-range, each partition row (fixed i)
    #    is contiguous in DRAM -> DMA descriptors are multi-KB runs
    x_dram = nc.dram_tensor("x", [2, B, D, NT, NB], F16, kind="ExternalInput")
    # w: chunk (kv, tq) holds lhsT tiles for taps [tq*TQ, tq*TQ+TQ) of the
    #    first conv half (Gf) followed by the same taps of the second (Gs)
    w_dram = nc.dram_tensor("w", [2, NQ, D, 2 * TQ * D], F16, kind="ExternalInput")
    bias_dram = nc.dram_tensor("bias", [D, 2], F32, kind="ExternalInput")
    # out: [kv, seq, o, chunk]
    out_dram = nc.dram_tensor("out", [2, B, D, NCH], F32, kind="ExternalOutput")

    # DMA ring discipline: nc.sync ring carries the x stream and the weight
    # chunks in exact consumption order (HWDGE completes FIFO per ring, so
    # the next-needed slab always lands first); bias/stores on nc.scalar.
    with tile.TileContext(nc) as tc:
        with (
            tc.tile_pool(name="sbw", bufs=1) as sbw,
            tc.tile_pool(name="sbx", bufs=x_bufs) as sbx,
            tc.tile_pool(name="sbo", bufs=6) as sbo,
            tc.tile_pool(name="ps", bufs=3, space=bass.MemorySpace.PSUM) as ps,
        ):
            bias_sb = sbw.tile([D, 2], F32)
            nc.scalar.dma_start(bias_sb[:], bias_dram[:])

            # weight chunks ride the sync ring just-in-time, in consumption
            # order ahead of the x unit that first needs them
            w_sb = {}

            def ensure_w(c, tqc):
                if (c, tqc) in w_sb:
                    return w_sb[(c, tqc)]
                wt = sbw.tile([D, 2 * TQ * D], F16, name=f"w_sb_{c}_{tqc}")
                w_sb[(c, tqc)] = wt
                nc.sync.dma_start(wt[:], w_dram[c, tqc])
                return wt

            for c in range(2):
                for s in range(B):
                    gf = ps.tile([D, NB], F32, tag="gf", name=f"gf_{c}_{s}")
                    gs = ps.tile([D, NB], F32, tag="gs", name=f"gs_{c}_{s}")
                    # ramp the very first unit so the PE starts early
                    if c == 0 and s == 0:
                        ranges = [(0, 2), (2, 4), (4, 8), (8, 16)]
                    else:
                        ranges = [(0, TQ), (TQ, 2 * TQ)][:NT // TQ]
                    for (ta, tb) in ranges:
                        tqc = ta // TQ
                        wid = tb - ta
                        wt = ensure_w(c, tqc)
                        xt = sbx.tile([D, wid, NB], F16, tag=f"x{wid}",
                                      bufs=(None if wid == TQ else 2),
                                      name=f"x_{c}_{s}_{ta}")
                        nc.sync.dma_start(xt[:], x_dram[c, s, :, ta:tb, :])
                        for lt in range(wid):
                            t = ta + lt
                            wo = t - tqc * TQ
                            nc.tensor.matmul(
                                gf[:], wt[:, wo * D:(wo + 1) * D], xt[:, lt, :],
                                start=(t == 0), stop=(t == NT - 1),
                            )
                            nc.tensor.matmul(
                                gs[:], wt[:, (TQ + wo) * D:(TQ + wo + 1) * D],
                                xt[:, lt, :],
                                start=(t == 0), stop=(t == NT - 1),
                            )
                    # out[:, l] = Gf[:, l] + Gs[:, l+1] + bias, l in [0, 511)
                    t1 = sbo.tile([D, NCH], F32, tag="t1", name=f"t1_{c}_{s}")
                    nc.scalar.activation(
                        t1[:], gf[:, 0:NCH],
                        mybir.ActivationFunctionType.Identity,
                        bias=bias_sb[:, c:c + 1],
                    )
                    ot = sbo.tile([D, NCH], F32, tag="ot", name=f"ot_{c}_{s}")
                    nc.vector.tensor_add(ot[:], t1[:], gs[:, 1:NB])
                    nc.scalar.dma_start(out_dram[c, s], ot[:])

    nc.compile()
    return nc


def _get_nc():
    if "nc" not in _compiled:
        _compiled["nc"] = _build()
    return _compiled["nc"]


def kernel(kv, cu_seqlens, k_conv_w, k_conv_b, v_conv_w, v_conv_b):
    global last_results
    kv = np.ascontiguousarray(np.asarray(kv, dtype=np.float32))
    k_conv_w = np.asarray(k_conv_w, dtype=np.float32)
    v_conv_w = np.asarray(v_conv_w, dtype=np.float32)
    k_conv_b = np.asarray(k_conv_b, dtype=np.float32)
    v_conv_b = np.asarray(v_conv_b, dtype=np.float32)

    nc = _get_nc()

    # x view: [s, b, t, c, h, i]
    a = kv.reshape(B, NB, NT, 2, H, D)
    # weights: [o, i, t] -> [c, i, t, o] -> per-quarter chunks
    # chunk (c, tq) = taps [tq*TQ, (tq+1)*TQ) of half 0 ++ same taps of half 1
    wf = np.stack([k_conv_w, v_conv_w]).transpose(0, 2, 3, 1)  # [2, i, t, o]
    w_all = np.ascontiguousarray(
        np.stack(
            [
                np.concatenate(
                    [
                        wf[:, :, tq * TQ:(tq + 1) * TQ, :],
                        wf[:, :, NT + tq * TQ:NT + (tq + 1) * TQ, :],
                    ],
                    axis=2,
                )
                for tq in range(NQ)
            ],
            axis=1,
        )
    ).reshape(2, NQ, D, 2 * TQ * D).astype(np.float16)  # [2, tq, i, (tap, o)]
    bias_all = np.ascontiguousarray(
        np.stack([k_conv_b, v_conv_b], axis=1)
    )  # [o, 2]

    in_maps = []
    for h in range(H):
        xh = np.ascontiguousarray(
            a[:, :, :, :, h, :].transpose(3, 0, 4, 2, 1).astype(np.float16)
        )  # [c, s, i, t, b]
        in_maps.append({"x": xh, "w": w_all, "bias": bias_all})

    res = bass_utils.run_bass_kernel_spmd(
        nc, in_maps, core_ids=list(range(H)),
    )
    last_results = res

    compress_k = np.empty((L, H, D), dtype=np.float32)
    compress_v = np.empty((L, H, D), dtype=np.float32)
    for h in range(H):
        out = res.results[h]["out"]  # [2, B, D, NCH]
        compress_k[:, h, :] = out[0].transpose(0, 2, 1).reshape(L, D)
        compress_v[:, h, :] = out[1].transpose(0, 2, 1).reshape(L, D)
    cu_comp = np.arange(B + 1, dtype=np.int32) * NCH
    return compress_k, compress_v, cu_comp


# revision 29
# speedup vs baseline: 1.3426x; 1.0649x over previous
"""CompressKV Trainium2 kernel.

Reference computation (see problem):
  kv [32768, 2, 8, 128] fp32, 4 sequences of 8192 tokens.
  Each sequence is cut into 511 chunks of 32 tokens, stride 16.
  compress_k[l,h,o] = sum_{t,i} k[chunk l, t, h, i] * k_conv_w[o,i,t] + k_conv_b[o]
  (same for v), returns (compress_k, compress_v, cu_comp).

Strategy:
  - Head-sharded across the 8 NeuronCores (weights are shared across heads,
    each core handles one head for both K and V over all 4 sequences).
  - Chunks overlap by half (kernel 32, stride 16) -> block decomposition:
    block b = tokens [16b, 16b+16); out[l] = Gf[l] + Gs[l+1] where
    Gf[b] = W[:, :, 0:16] . block_b and Gs[b] = W[:, :, 16:32] . block_b.
    This way each token is streamed from HBM exactly once.
  - Per (kv, seq): 32 accumulating matmuls of [K=128 (=head_dim), M=128 (=out)]
    x [K=128, N=512 blocks] into two PSUM banks (Gf, Gs). Host pre-transposes
    the data so every DMA and every matmul access is contiguous.
  - x and w stream as fp16: the PE computes fp16 products exactly with fp32
    PSUM accumulation (verified on HW), and fp16's 11-bit mantissa matches
    what the fp32 "replicated" (float32r) matmul mode keeps internally
    anyway — so this halves DMA bytes at near-identical accuracy.
    Measured end-to-end L2 relative error ~2.8e-4 vs the fp32 reference.
  - DMA ring discipline: HWDGE completes FIFO per ring, so the sync ring
    carries x slabs + weight chunks in exact consumption order (the
    next-needed slab always lands first); bias/stores ride the scalar ring.
    The first (kv, seq) unit is split 2/2/4/8 taps so the PE starts early.
  - Shift-add (Gf[l] + Gs[l+1]) + bias done on ACT + DVE, exact fp32.
  - ~79-84 us per-core HW exec: preamble ~7, DMA-fed matmul stream ~60
    (stack-bandwidth bound, zero PE gaps), drain tail ~8.
"""
import sys

sys.path.insert(0, "/opt/trn_rl_repo")

import numpy as np

from concourse import bass, bacc, tile, bass_utils, mybir

B = 4          # sequences
SEQ = 8192     # tokens per sequence
H = 8          # heads (== number of cores)
D = 128        # head dim
KS = 32        # chunk size
STRIDE = 16    # chunk stride
NB = SEQ // STRIDE          # 512 blocks per sequence
NCH = (SEQ - KS) // STRIDE + 1  # 511 chunks per sequence
NT = STRIDE    # 16 K-tiles per conv half
L = B * NCH    # 2044 chunks total

F32 = mybir.dt.float32
F32R = mybir.dt.float32r
F16 = mybir.dt.float16

_compiled = {}

# results of the last hardware run (for test harness introspection)
last_results = None


TQ = 8           # conv taps per x DMA unit (1 MB per transfer)
NQ = NT // TQ    # 4 quarters per (kv, seq)


def _build(x_bufs=13):
    nc = bacc.Bacc("TRN2", target_bir_lowering=False, debug=False, num_devices=H)

    # per-core inputs (core h owns head h)
    # x: [kv, seq, i, t', block]: for any t'-range, each partition row (fixed i)
    #    is contiguous in DRAM -> DMA descriptors are multi-KB runs
    x_dram = nc.dram_tensor("x", [2, B, D, NT, NB], F16, kind="ExternalInput")
    # w: chunk (kv, tq) holds lhsT tiles for taps [tq*TQ, tq*TQ+TQ) of the
    #    first conv half (Gf) followed by the same taps of the second (Gs)
    w_dram = nc.dram_tensor("w", [2, NQ, D, 2 * TQ * D], F16, kind="ExternalInput")
    bias_dram = nc.dram_tensor("bias", [D, 2], F32, kind="ExternalInput")
    # out: [kv, seq, o, chunk]
    out_dram = nc.dram_tensor("out", [2, B, D, NCH], F32, kind="ExternalOutput")

    # DMA ring discipline: nc.sync ring carries the x stream and the weight
    # chunks in exact consumption order (HWDGE completes FIFO per ring, so
    # the next-needed slab always lands first); bias/stores on nc.scalar.
    with tile.TileContext(nc) as tc:
        with (
            tc.tile_pool(name="sbw", bufs=1) as sbw,
            tc.tile_pool(name="sbx", bufs=x_bufs) as sbx,
            tc.tile_pool(name="sbo", bufs=6) as sbo,
            tc.tile_pool(name="ps", bufs=3, space=bass.MemorySpace.PSUM) as ps,
        ):
            bias_sb = sbw.tile([D, 2], F32)
            nc.scalar.dma_start(bias_sb[:], bias_dram[:])

            # weight chunks ride the sync ring just-in-time, in consumption
            # order ahead of the x unit that first needs them
            w_sb = {}

            def ensure_w(c, tqc):
                if (c, tqc) in w_sb:
                    return w_sb[(c, tqc)]
                wt = sbw.tile([D, 2 * TQ * D], F16, name=f"w_sb_{c}_{tqc}")
                w_sb[(c, tqc)] = wt
                nc.sync.dma_start(wt[:], w_dram[c, tqc])
                return wt

            for c in range(2):
                for s in range(B):
                    gf = ps.tile([D, NB], F32, tag="gf", name=f"gf_{c}_{s}")
                    gs = ps.tile([D, NB], F32, tag="gs", name=f"gs_{c}_{s}")
                    # ramp the very first unit so the PE starts early
                    if c == 0 and s == 0:
                        ranges = [(0, 2), (2, 4), (4, 8), (8, 16)]
                    else:
                        ranges = [(0, TQ), (TQ, 2 * TQ)][:NT // TQ]
                    for (ta, tb) in ranges:
                        tqc = ta // TQ
                        wid = tb - ta
                        wt = ensure_w(c, tqc)
                        xt = sbx.tile([D, wid, NB], F16, tag=f"x{wid}",
                                      bufs=(None if wid == TQ else 2),
                                      name=f"x_{c}_{s}_{ta}")
                        nc.sync.dma_start(xt[:], x_dram[c, s, :, ta:tb, :])
                        for lt in range(wid):
                            t = ta + lt
                            wo = t - tqc * TQ
                            nc.tensor.matmul(
                                gf[:], wt[:, wo * D:(wo + 1) * D], xt[:, lt, :],
                                start=(t == 0), stop=(t == NT - 1),
                            )
                            nc.tensor.matmul(
                                gs[:], wt[:, (TQ + wo) * D:(TQ + wo + 1) * D],
                                xt[:, lt, :],
                                start=(t == 0), stop=(t == NT - 1),
                            )
                    # out[:, l] = Gf[:, l] + Gs[:, l+1] + bias, l in [0, 511)
                    t1 = sbo.tile([D, NCH], F32, tag="t1", name=f"t1_{c}_{s}")
                    nc.scalar.activation(
                        t1[:], gf[:, 0:NCH],
                        mybir.ActivationFunctionType.Identity,
                        bias=bias_sb[:, c:c + 1],
                    )
                    ot = sbo.tile([D, NCH], F32, tag="ot", name=f"ot_{c}_{s}")
                    nc.vector.tensor_add(ot[:], t1[:], gs[:, 1:NB])
                    nc.scalar.dma_start(out_dram[c, s], ot[:])

    nc.compile()
    return nc


def _get_nc():
    if "nc" not in _compiled:
        _compiled["nc"] = _build()
    return _compiled["nc"]


def kernel(kv, cu_seqlens, k_conv_w, k_conv_b, v_conv_w, v_conv_b):
    global last_results
    kv = np.ascontiguousarray(np.asarray(kv, dtype=np.float32))
    k_conv_w = np.asarray(k_conv_w, dtype=np.float32)
    v_conv_w = np.asarray(v_conv_w, dtype=np.float32)
    k_conv_b = np.asarray(k_conv_b, dtype=np.float32)
    v_conv_b = np.asarray(v_conv_b, dtype=np.float32)

    nc = _get_nc()

    # x view: [s, b, t, c, h, i]
    a = kv.reshape(B, NB, NT, 2, H, D)
    # weights: [o, i, t] -> [c, i, t, o] -> per-quarter chunks
    # chunk (c, tq) = taps [tq*TQ, (tq+1)*TQ) of half 0 ++ same taps of half 1
    wf = np.stack([k_conv_w, v_conv_w]).transpose(0, 2, 3, 1)  # [2, i, t, o]
    w_all = np.ascontiguousarray(
        np.stack(
            [
                np.concatenate(
                    [
                        wf[:, :, tq * TQ:(tq + 1) * TQ, :],
                        wf[:, :, NT + tq * TQ:NT + (tq + 1) * TQ, :],
                    ],
                    axis=2,
                )
                for tq in range(NQ)
            ],
            axis=1,
        )
    ).reshape(2, NQ, D, 2 * TQ * D).astype(np.float16)  # [2, tq, i, (tap, o)]
    bias_all = np.ascontiguousarray(
        np.stack([k_conv_b, v_conv_b], axis=1)
    )  # [o, 2]

    in_maps = []
    for h in range(H):
        xh = np.ascontiguousarray(
            a[:, :, :, :, h, :].transpose(3, 0, 4, 2, 1).astype(np.float16)
        )  # [c, s, i, t, b]
        in_maps.append({"x": xh, "w": w_all, "bias": bias_all})

    res = bass_utils.run_bass_kernel_spmd(
        nc, in_maps, core_ids=list(range(H)),
    )
    last_results = res

    compress_k = np.empty((L, H, D), dtype=np.float32)
    compress_v = np.empty((L, H, D), dtype=np.float32)
    for h in range(H):
        out = res.results[h]["out"]  # [2, B, D, NCH]
        compress_k[:, h, :] = out[0].transpose(0, 2, 1).reshape(L, D)
        compress_v[:, h, :] = out[1].transpose(0, 2, 1).reshape(L, D)
    cu_comp = np.arange(B + 1, dtype=np.int32) * NCH
    return compress_k, compress_v, cu_comp
